# revision 2
# baseline (speedup 1.0000x reference)
"""3-layer GCN (gcn_norm message passing) on 8 Trainium2 NeuronCores — v5.

Architecture (v5, wave-pipelined):
  - Nodes row-sharded across 8 cores (12500 real + 44 pad rows each). Per
    layer the bf16 message table hm = (relu(h) @ W) * dis[src] is exchanged
    in FOUR quarter AllGathers (source-blocks 0-23 / 24-47 / 48-71 / 72-97),
    each issued eagerly as soon as the previous layer's epilogue finishes
    producing that quarter's blocks — the collective hides behind gathers.
  - Messages (real edges only; self-loops are handled on-chip) are sorted by
    (dest block, source quarter); per-(block, quarter) run capacities are the
    exact max across cores (RUN=1). Each (group, wave) gather is split into
    4 sub-calls, one per SWDGE queue, so all 4 queues stay busy within a
    single wave (wave w gathers from quarter-w's AllGathered window).
  - Segment-sum on the TensorEngine: per chunk of 128 gathered messages one
    matmul per overlapped dest block into a [128, 512] group PSUM tile;
    one-hots are bf16 iota==meta compares (2x DVE rate), built 8 chunks per
    wide op from the SBUF-resident meta. Wave results accumulate into an
    SBUF f32 accumulator; wave 3 also adds the self-loop contribution via
    identity matmuls reading the locally retained hm blocks.
  - Epilogue per group: ob = acc * dis[dest] (DVE) + bias (ACT), bf16 store
    to h_out; relu (ACT), next-layer matmul (PE), hm = psum * dis (ACT) into
    the persistent hm_keep tile + DMA to the next layer's ag_in quarters.
  - gidx / meta / disd / iota live in SBUF for the whole kernel (no per-group
    reloads), minimizing HWDGE traffic that would otherwise dilute the
    SWDGE queues' packet rate.

All data-dependent structure is baked at trace time; the NEFF is compiled
per call and cached in-process.
"""

import os
import sys

sys.path.insert(0, "/opt/trn_rl_repo")

import numpy as np
import ml_dtypes

from concourse import bacc, bass, mybir
from concourse import tile
from concourse import bass_utils

F32 = mybir.dt.float32
BF16 = mybir.dt.bfloat16
I16 = mybir.dt.int16
BFNP = ml_dtypes.bfloat16

N_CORES = 8
NQ = 4       # source quarters == waves == SWDGE queues
G = 4        # dest blocks per group
WOH = 8      # one-hot chunks per wide DVE op
PAD_SEG = 30000.0


def _quarter_bounds(nblk):
    """Split nblk dest/source blocks into 4 quarters at group granularity."""
    ngrp = (nblk + G - 1) // G
    base = ngrp // NQ
    ext = ngrp % NQ
    qg = []
    g0 = 0
    for w in range(NQ):
        n = base + (1 if w >= NQ - ext else 0)
        qg.append((g0, g0 + n))
        g0 += n
    # block bounds per quarter
    qb = [(min(a * G, nblk), min(b * G, nblk)) for a, b in qg]
    return qg, qb


def _schedule(caps, nblk):
    """caps: [nblk][NQ] per-(block, quarter) run capacity in slots.

    Returns the full static layout: per-(group, wave) sub-call split across
    the 4 queues, gidx column bases, chunk->mm schedule, mm column bases.
    """
    ngrp = (nblk + G - 1) // G
    qg, qb = _quarter_bounds(nblk)

    run_slot = {}           # (b, w) -> slot offset inside call (g, w)
    call_parts = {}         # (g, w) -> list of (c0, c1) chunk ranges
    part_col = {}           # (g, w, part) -> gidx col base
    mm_of = {}              # (g, w) -> list of (j, c) chunk mms, j-major
    mm_base = {}            # (g, w) -> global mm col base
    nch_of = {}
    col_ctr = 0
    mm_ctr = 0
    for w in range(NQ):
        for g in range(ngrp):
            blocks = list(range(g * G, min((g + 1) * G, nblk)))
            off = 0
            spans = []
            for b in blocks:
                run_slot[(b, w)] = off
                spans.append((b - g * G, off, off + caps[b][w]))
                off += caps[b][w]
            nch = (off + 127) // 128
            nch_of[(g, w)] = nch
            # one call per (g, w); queue = g % NQ keeps all queues busy
            parts = [(0, nch)]
            part_col[(g, w, 0)] = col_ctr
            col_ctr += nch * 8  # 128 slots/chunk / 16 rows
            call_parts[(g, w)] = parts
            mms = []
            for c in range(nch):
                lo, hi = c * 128, (c + 1) * 128
                for j, s0, s1 in spans:
                    if s0 < hi and s1 > lo:
                        mms.append((j, c))
            mms.sort()
            mm_of[(g, w)] = mms
            mm_base[(g, w)] = mm_ctr
            mm_ctr += len(mms)
    return {
        "ngrp": ngrp, "qg": qg, "qb": qb,
        "run_slot": run_slot, "call_parts": call_parts,
        "part_col": part_col, "mm_of": mm_of, "mm_base": mm_base,
        "nch_of": nch_of,
        "gidx_cols": col_ctr, "n_mm": mm_ctr,
    }


# ----------------------------------------------------------------------------
# Host-side preparation
# ----------------------------------------------------------------------------

def _prep_inputs(x, edge_index, W0, b0, W1, b1, W2, b2, s_real):
    n = x.shape[0]
    assert n % N_CORES == 0 and s_real == n // N_CORES
    nblk = (s_real + 127) // 128
    s_pad = nblk * 128
    ngrp = (nblk + G - 1) // G
    qg, qb = _quarter_bounds(nblk)
    qrows = [(b1_ - b0_) * 128 for b0_, b1_ in qb]
    assert all(N_CORES * r <= 32767 for r in qrows)
    quarter_of_block = np.zeros(nblk, dtype=np.int64)
    for w, (b0_, b1_) in enumerate(qb):
        quarter_of_block[b0_:b1_] = w
    qstart = np.array([b0_ * 128 for b0_, _ in qb], dtype=np.int64)
    qrows_a = np.array(qrows, dtype=np.int64)

    d = np.asarray(edge_index[0], dtype=np.int64)
    s = np.asarray(edge_index[1], dtype=np.int64)

    deg = np.bincount(s, minlength=n).astype(np.float64) + 1.0
    dis = (1.0 / np.sqrt(deg)).astype(np.float32)

    core_d = d // s_real
    dloc = d - core_d * s_real
    blk = dloc >> 7
    grp = blk >> 2
    jj = blk & 3

    core_s = s // s_real
    sloc = s - core_s * s_real
    sblk = sloc >> 7
    w_e = quarter_of_block[sblk]
    widx = core_s * qrows_a[w_e] + (sloc - qstart[w_e])

    key = (core_d * nblk + blk) * NQ + w_e
    counts = np.bincount(key, minlength=N_CORES * nblk * NQ).reshape(
        N_CORES, nblk, NQ
    )
    caps = np.maximum(counts.max(axis=0), 1)  # [nblk, NQ], RUN=1

    lay = _schedule(caps.tolist(), nblk)

    order = np.argsort(key, kind="stable")
    inv = np.empty_like(order)
    inv[order] = np.arange(order.size)
    starts = np.zeros(N_CORES * nblk * NQ + 1, dtype=np.int64)
    np.cumsum(counts.reshape(-1), out=starts[1:])
    rank = inv - starts[key]

    run_slot_arr = np.zeros((nblk, NQ), dtype=np.int64)
    for (b, w), v in lay["run_slot"].items():
        run_slot_arr[b, w] = v

    slot = run_slot_arr[blk, w_e] + rank        # slot within call (g, w)
    chunk = slot >> 7

    # part of chunk + gidx col
    npgw = lay["ngrp"] * NQ
    part_lo = np.zeros((lay["ngrp"], NQ, 1), dtype=np.int64)   # c0 per part
    part_colbase = np.zeros((lay["ngrp"], NQ, 1), dtype=np.int64)
    for (g, w), parts in lay["call_parts"].items():
        for p, (c0, c1) in enumerate(parts):
            part_lo[g, w, p] = c0
            part_colbase[g, w, p] = lay["part_col"][(g, w, p)]
    # map chunk -> part via searchsorted per (g, w)
    part_of = np.zeros_like(chunk)
    for (g, w), parts in lay["call_parts"].items():
        m = (grp == g) & (w_e == w)
        if not m.any():
            continue
        bounds = [c1 for _, c1 in parts[:-1]]
        part_of[m] = np.searchsorted(bounds, chunk[m], side="right")
    gcol = (part_colbase[grp, w_e, part_of]
            + ((slot - part_lo[grp, w_e, part_of] * 128) >> 4))
    grow = slot & 15

    gidx16 = np.zeros((N_CORES, 16, lay["gidx_cols"]), dtype=np.int16)
    gidx16[core_d, grow, gcol] = widx.astype(np.int16)
    gidx = np.broadcast_to(
        gidx16[:, None, :, :], (N_CORES, 8, 16, lay["gidx_cols"])
    ).reshape(N_CORES, 128, lay["gidx_cols"]).copy()

    # mm col lookup: (g, w, chunk, j) -> global mm column
    mm_col = {}
    for (g, w), mms in lay["mm_of"].items():
        m0 = lay["mm_base"][(g, w)]
        for k, (j, c) in enumerate(mms):
            mm_col[(g, w, c, j)] = m0 + k
    mm_col_arr = np.full((lay["ngrp"], NQ, 64, G), -1, dtype=np.int64)
    for (g, w, c, j), v in mm_col.items():
        mm_col_arr[g, w, c, j] = v

    meta = np.full((N_CORES, 128, lay["n_mm"]), PAD_SEG, dtype=np.float32)
    col = mm_col_arr[grp, w_e, chunk, jj]
    assert (col >= 0).all()
    meta[core_d, slot & 127, col] = (dloc & 127).astype(np.float32)

    # dense inputs
    x = np.asarray(x, dtype=np.float32)
    x_t = np.zeros((N_CORES, 128, s_pad), dtype=BFNP)
    dison = np.zeros((N_CORES, 128, nblk), dtype=np.float32)
    disd = np.zeros((N_CORES, 128, s_pad), dtype=BFNP)
    for r in range(N_CORES):
        x_t[r, :, :s_real] = x[r * s_real : (r + 1) * s_real].T.astype(BFNP)
        dv = np.zeros(s_pad, dtype=np.float32)
        dv[:s_real] = dis[r * s_real : (r + 1) * s_real]
        dison[r] = dv.reshape(nblk, 128).T
        disd[r] = dv[None, :].astype(BFNP)

    wdata = np.zeros((128, 3 * 128 + 3), dtype=np.float32)
    wdata[:, 0:128] = np.asarray(W0, dtype=np.float32)
    wdata[:, 128:256] = np.asarray(W1, dtype=np.float32)
    wdata[:, 256:384] = np.asarray(W2, dtype=np.float32)
    wdata[:, 384] = np.asarray(b0, dtype=np.float32)
    wdata[:, 385] = np.asarray(b1, dtype=np.float32)
    wdata[:, 386] = np.asarray(b2, dtype=np.float32)
    iotar = np.tile(np.arange(128, dtype=np.float32), WOH)[None, :].repeat(
        128, axis=0
    ).astype(BFNP)
    ident = np.eye(128, dtype=np.float32).astype(BFNP)

    in_maps = [
        {
            "x_t": x_t[r],
            "meta": meta[r].astype(BFNP),
            "gidx": gidx[r],
            "wdata": wdata, "iotar": iotar, "ident": ident,
            "dison": dison[r], "disd": disd[r],
        }
        for r in range(N_CORES)
    ]
    sched = {
        "nblk": nblk, "s_pad": s_pad, "s_real": s_real,
        "caps": caps.tolist(),
    }
    return in_maps, sched


# ----------------------------------------------------------------------------
# Device kernel builder
# ----------------------------------------------------------------------------

def build_kernel(sched, n_cores=N_CORES):
    from contextlib import ExitStack

    nblk, s_pad = sched["nblk"], sched["s_pad"]
    caps = sched["caps"]
    lay = _schedule(caps, nblk)
    ngrp = lay["ngrp"]
    qg, qb = lay["qg"], lay["qb"]
    qrows = [(b1_ - b0_) * 128 for b0_, b1_ in qb]
    quarter_of_block = []
    for b in range(nblk):
        for w, (b0_, b1_) in enumerate(qb):
            if b0_ <= b < b1_:
                quarter_of_block.append(w)
                break
    qlast_grp = [b - 1 for _, b in qg]  # last group index of each quarter

    nc = bacc.Bacc(
        "TRN2", target_bir_lowering=False, debug=False, num_devices=n_cores,
        num_swdge_queues=NQ,
    )
    x_t = nc.dram_tensor("x_t", [128, s_pad], BF16, kind="ExternalInput")
    meta = nc.dram_tensor("meta", [128, lay["n_mm"]], BF16, kind="ExternalInput")
    gidx = nc.dram_tensor("gidx", [128, lay["gidx_cols"]], I16, kind="ExternalInput")
    wdata = nc.dram_tensor("wdata", [128, 3 * 128 + 3], F32, kind="ExternalInput")
    iotar = nc.dram_tensor("iotar", [128, WOH * 128], BF16, kind="ExternalInput")
    identt = nc.dram_tensor("ident", [128, 128], BF16, kind="ExternalInput")
    dison = nc.dram_tensor("dison", [128, nblk], F32, kind="ExternalInput")
    disd = nc.dram_tensor("disd", [128, s_pad], BF16, kind="ExternalInput")
    h_out = nc.dram_tensor("h_out", [128, 3 * s_pad], BF16, kind="ExternalOutput")

    rg = [list(range(n_cores))]
    ID = mybir.ActivationFunctionType

    with tile.TileContext(nc) as tc, ExitStack() as ctx:
        const = ctx.enter_context(tc.tile_pool(name="const", bufs=1))
        xw = ctx.enter_context(tc.tile_pool(name="xw", bufs=4))
        dram = ctx.enter_context(tc.tile_pool(name="dram", bufs=1, space="DRAM"))
        gath = ctx.enter_context(tc.tile_pool(name="gath", bufs=2 * NQ))
        ohp = ctx.enter_context(tc.tile_pool(name="ohp", bufs=8))
        outsb = ctx.enter_context(tc.tile_pool(name="outsb", bufs=2))
        rsb = ctx.enter_context(tc.tile_pool(name="rsb", bufs=2))
        agg_ps = ctx.enter_context(tc.tile_pool(name="agg_ps", bufs=2, space="PSUM"))
        mm_ps = ctx.enter_context(tc.tile_pool(name="mm_ps", bufs=2, space="PSUM"))
        mma_ps = ctx.enter_context(tc.tile_pool(name="mma_ps", bufs=2, space="PSUM"))

        ag_in = [[dram.tile([qrows[w], 128], BF16, name=f"ag_in_{p}_{w}")
                  for w in range(NQ)] for p in range(2)]
        ag_out = [[dram.tile([n_cores * qrows[w], 128], BF16,
                             addr_space="Shared", name=f"ag_out_{l}_{w}")
                   for w in range(NQ)] for l in range(3)]

        w_sb = const.tile([128, 3 * 128 + 3], F32)
        nc.sync.dma_start(out=w_sb[:], in_=wdata[:])
        w_bf = const.tile([128, 3 * 128], BF16)
        nc.vector.tensor_copy(w_bf[:], w_sb[:, 0 : 3 * 128])
        iota_sb = const.tile([128, WOH * 128], BF16)
        nc.sync.dma_start(out=iota_sb[:], in_=iotar[:])
        ident_sb = const.tile([128, 128], BF16)
        nc.sync.dma_start(out=ident_sb[:], in_=identt[:])
        dison_sb = const.tile([128, nblk], F32)
        nc.sync.dma_start(out=dison_sb[:], in_=dison[:])
        disd_sb = const.tile([128, s_pad], BF16)
        nc.sync.dma_start(out=disd_sb[:], in_=disd[:])
        meta_sb = const.tile([128, lay["n_mm"]], BF16)
        nc.sync.dma_start(out=meta_sb[:], in_=meta[:])
        gidx_sb = const.tile([128, lay["gidx_cols"]], I16)
        nc.sync.dma_start(out=gidx_sb[:], in_=gidx[:])

        hm_keep = const.tile([128, s_pad], BF16)   # [node-in-block, feat] per block
        acc = const.tile([128, s_pad], F32)        # [feat, dest]

        def bias(L):
            return w_sb[:, 384 + L : 385 + L]

        def issue_ag(L, w):
            nc.gpsimd.collective_compute(
                "AllGather",
                mybir.AluOpType.bypass,
                replica_groups=rg,
                ins=[ag_in[L % 2][w][:].opt()],
                outs=[ag_out[L][w][:].opt()],
            )

        # ---- Phase A: table0 = (x @ W0) * dis -> ag_in[0] + hm_keep ----
        for b in range(nblk):
            xt = xw.tile([128, 128], BF16)
            nc.sync.dma_start(out=xt[:], in_=x_t[:, b * 128 : (b + 1) * 128])
            ps = mma_ps.tile([128, 128], F32, name="psA", tag="psA")
            nc.tensor.matmul(
                ps[:], lhsT=xt[:], rhs=w_bf[:, 0:128], start=True, stop=True
            )
            nc.scalar.activation(
                hm_keep[:, b * 128 : (b + 1) * 128], ps[:], ID.Copy,
                scale=dison_sb[:, b : b + 1],
            )
            w = quarter_of_block[b]
            r0 = (b - qb[w][0]) * 128
            nc.scalar.dma_start(
                out=ag_in[0][w][r0 : r0 + 128, :],
                in_=hm_keep[:, b * 128 : (b + 1) * 128],
            )
            if b == qb[w][1] - 1:
                issue_ag(0, w)

        # ---- 3 layers ----
        # phase I: wave 0 for all groups (gated only by AG_0); phase II:
        # group-major waves 1-3 with inline epilogues so the next layer's
        # AllGathers are issued from ~30% of the layer onward.
        for L in range(3):
            p = L % 2
            for w, g in ([(0, g) for g in range(ngrp)]
                         + [(w, g) for g in range(ngrp) for w in (1, 2)]
                         + [(3, g) for g in range(ngrp)]):
                if True:
                    blocks = list(range(g * G, min((g + 1) * G, nblk)))
                    nj = len(blocks)
                    parts = lay["call_parts"][(g, w)]
                    gts = []
                    for pi, (c0, c1) in enumerate(parts):
                        nidx = (c1 - c0) * 128
                        if nidx == 0:
                            gts.append(None)
                            continue
                        cb = lay["part_col"][(g, w, pi)]
                        gt = gath.tile([128, nidx], BF16, name="gt", tag="gt")
                        nc.gpsimd.dma_gather(
                            gt[:].rearrange("p (c f) -> p c f", f=128),
                            ag_out[L][w][:],
                            gidx_sb[:, cb : cb + nidx // 16],
                            num_idxs=nidx,
                            num_idxs_reg=nidx,
                            elem_size=128,
                            elem_step=128,
                            single_packet=False,
                            queue_num=g % NQ,
                        )
                        gts.append(gt)

                    mms = lay["mm_of"][(g, w)]
                    m0 = lay["mm_base"][(g, w)]
                    n_mm_g = len(mms)
                    ohs = {}
                    for w0 in range(0, n_mm_g, WOH):
                        wn = min(WOH, n_mm_g - w0)
                        oh = ohp.tile([128, wn * 128], BF16, name="oh", tag="oh")
                        nc.vector.tensor_tensor(
                            oh[:].rearrange("p (c f) -> p c f", f=128),
                            iota_sb[:, : wn * 128].rearrange(
                                "p (c f) -> p c f", f=128
                            ),
                            meta_sb[:, m0 + w0 : m0 + w0 + wn].to_broadcast(
                                [128, wn, 128]
                            ),
                            mybir.AluOpType.is_equal,
                        )
                        ohs[w0] = oh

                    ps = agg_ps.tile([128, G * 128], F32, name="aggps", tag="aggps")
                    # j-major schedule; wave 3 prepends identity (self-loop) mms
                    sched_mms = []
                    if w == NQ - 1:
                        for j in range(nj):
                            sched_mms.append((j, "ident"))
                    for k, (j, c) in enumerate(mms):
                        sched_mms.append((j, k))
                    sched_mms.sort(key=lambda t: (t[0], isinstance(t[1], int),
                                                  t[1] if isinstance(t[1], int) else -1))
                    firsts = {}
                    lasts = {}
                    for idx_, (j, kk) in enumerate(sched_mms):
                        if j not in firsts:
                            firsts[j] = idx_
                        lasts[j] = idx_
                    for idx_, (j, kk) in enumerate(sched_mms):
                        st = idx_ == firsts[j]
                        sp = idx_ == lasts[j]
                        if kk == "ident":
                            b = blocks[j]
                            nc.tensor.matmul(
                                ps[:, j * 128 : (j + 1) * 128],
                                lhsT=hm_keep[:, b * 128 : (b + 1) * 128],
                                rhs=ident_sb[:],
                                start=st, stop=sp,
                                skip_group_check=True,
                            )
                        else:
                            jj_, c = mms[kk]
                            # locate part + offset of chunk c
                            for pi, (c0, c1) in enumerate(parts):
                                if c0 <= c < c1:
                                    break
                            off = c - c0
                            w0 = (kk // WOH) * WOH
                            ohoff = kk - w0
                            nc.tensor.matmul(
                                ps[:, j * 128 : (j + 1) * 128],
                                lhsT=gts[pi][:, off * 128 : (off + 1) * 128],
                                rhs=ohs[w0][:, ohoff * 128 : (ohoff + 1) * 128],
                                start=st, stop=sp,
                                skip_group_check=True,
                            )

                    wd = nj * 128
                    gb = g * G * 128
                    if w == 0:
                        nc.vector.tensor_copy(acc[:, gb : gb + wd], ps[:, :wd])
                    else:
                        nc.vector.tensor_tensor(
                            acc[:, gb : gb + wd], acc[:, gb : gb + wd],
                            ps[:, :wd], mybir.AluOpType.add,
                        )

                    if w == NQ - 1:
                        # ---- group epilogue ----
                        ob = outsb.tile([128, G * 128], BF16, name="ob", tag="ob")
                        nc.vector.tensor_tensor(
                            ob[:, :wd], acc[:, gb : gb + wd],
                            disd_sb[:, gb : gb + wd], mybir.AluOpType.mult,
                        )
                        nc.scalar.activation(
                            ob[:, :wd], ob[:, :wd], ID.Identity, bias=bias(L)
                        )
                        nc.sync.dma_start(
                            out=h_out[:, L * s_pad + gb : L * s_pad + gb + wd],
                            in_=ob[:, :wd],
                        )
                        if L < 2:
                            r = rsb.tile([128, G * 128], BF16, name="r", tag="r")
                            nc.scalar.activation(r[:, :wd], ob[:, :wd], ID.Relu)
                            ps2 = mm_ps.tile([128, G * 128], F32, name="ps2", tag="ps2")
                            for j in range(nj):
                                nc.tensor.matmul(
                                    ps2[:, j * 128 : (j + 1) * 128],
                                    lhsT=r[:, j * 128 : (j + 1) * 128],
                                    rhs=w_bf[:, (L + 1) * 128 : (L + 2) * 128],
                                    start=True, stop=True,
                                    skip_group_check=True,
                                )
                            for j in range(nj):
                                b = blocks[j]
                                nc.scalar.activation(
                                    hm_keep[:, b * 128 : (b + 1) * 128],
                                    ps2[:, j * 128 : (j + 1) * 128], ID.Copy,
                                    scale=dison_sb[:, b : b + 1],
                                )
                            for j in range(nj):
                                b = blocks[j]
                                wb = quarter_of_block[b]
                                r0 = (b - qb[wb][0]) * 128
                                nc.scalar.dma_start(
                                    out=ag_in[(L + 1) % 2][wb][r0 : r0 + 128, :],
                                    in_=hm_keep[:, b * 128 : (b + 1) * 128],
                                )
                            for w2 in range(NQ):
                                if g == qlast_grp[w2]:
                                    issue_ag(L + 1, w2)

    nc.compile()
    return nc


_BUILD_CACHE = {}


def _get_kernel(sched):
    key = (
        sched["nblk"], sched["s_pad"],
        tuple(tuple(c) for c in sched["caps"]),
    )
    if key not in _BUILD_CACHE:
        _BUILD_CACHE[key] = build_kernel(sched)
    return _BUILD_CACHE[key]


# ----------------------------------------------------------------------------
# Entry point
# ----------------------------------------------------------------------------

def _run(x, edge_index, W0, b0, W1, b1, W2, b2, trace=False):
    n = int(np.asarray(x).shape[0])
    s_real = n // N_CORES
    in_maps, sched = _prep_inputs(
        x, edge_index, W0, b0, W1, b1, W2, b2, s_real
    )
    s_pad = sched["s_pad"]
    nc = _get_kernel(sched)
    res = bass_utils.run_bass_kernel_spmd(
        nc, in_maps, core_ids=list(range(N_CORES)), trace=trace
    )
    outs = []
    for L in range(3):
        h = np.concatenate(
            [
                np.asarray(
                    res.results[r]["h_out"][:, L * s_pad : L * s_pad + s_real]
                ).astype(np.float32)
                for r in range(N_CORES)
            ],
            axis=1,
        ).T
        outs.append(h)
    full = np.stack(outs, axis=1).astype(np.float32)
    return full, res


def kernel(**inputs):
    trace = os.environ.get("TRN_KERNEL_TRACE", "") == "1"
    out, res = _run(
        np.asarray(inputs["x"]),
        np.asarray(inputs["edge_index"]),
        np.asarray(inputs["W0"]),
        np.asarray(inputs["b0"]),
        np.asarray(inputs["W1"]),
        np.asarray(inputs["b1"]),
        np.asarray(inputs["W2"]),
        np.asarray(inputs["b2"]),
        trace=trace,
    )
    if trace and res.exec_time_ns is not None:
        print(f"HW exec time: {res.exec_time_ns} ns")
        if res.instructions_and_trace:
            print(f"trace: {res.instructions_and_trace[1]}")
    return out
